# revision 1
# baseline (speedup 1.0000x reference)
"""Trainium2 Bass kernel for nn_CharTaggerBiLSTM, 8-core SPMD, 3 launches.

L1 char LSTM: data-parallel over batch (16 sentences/core). Transposed
   layout (features-on-partitions), f32r matmuls; emits the masked last
   hidden state per word -> DRAM.
L2 word LSTM: one direction per core (cores 0-3 forward, 4-7 backward),
   32 sentences/core so each weight stream serves twice the rows.
   Direction is data: backward cores receive the char outputs with the
   sentence axis reversed on host and their outputs are un-reversed.
   x-part/bias matmuls for step s+1 are issued during step s's
   elementwise work to keep PE fed.
L3 MLP + log_softmax: data-parallel (16 sentences/core), bf16 GEMMs.

Host does embedding gather, weight reshapes, the two reshard steps, and
reassembly.
"""

import sys
import functools
from contextlib import ExitStack

sys.path.insert(0, "/opt/trn_rl_repo")

import numpy as np
import ml_dtypes
from concourse import bacc, bass, mybir, tile, bass_utils

BF_NP = ml_dtypes.bfloat16


B, S, Lc = 128, 128, 20
AB, E = 100, 64
Hc, H, OUT = 256, 512, 50
NCORE = 8
BL = B // NCORE            # sentences per core in L1/L3
FP = mybir.dt.float32
FR = mybir.dt.float32r
BF = mybir.dt.bfloat16
G4 = 4 * Hc
WG = 4 * H

Sig = mybir.ActivationFunctionType.Sigmoid
TanhF = mybir.ActivationFunctionType.Tanh
ReluF = mybir.ActivationFunctionType.Relu
ExpF = mybir.ActivationFunctionType.Exp
LnF = mybir.ActivationFunctionType.Ln
IdentF = mybir.ActivationFunctionType.Identity


def build_l1(bl=BL):
    """Char LSTM, data-parallel; writes lastT [2,128,nl] to DRAM."""
    nl = bl * S
    nc = bacc.Bacc("TRN2", target_bir_lowering=False, debug=False,
                   num_devices=NCORE)
    d_eT = nc.dram_tensor("eT", [Lc, E, nl], FR, kind="ExternalInput")
    d_lenrep = nc.dram_tensor("lenrep", [128, nl], FP, kind="ExternalInput")
    d_cWxT = nc.dram_tensor("cWxT", [E, G4], FR, kind="ExternalInput")
    d_cWhT = nc.dram_tensor("cWhT", [2, 128, G4], FR, kind="ExternalInput")
    d_cbias = nc.dram_tensor("cbias", [128, G4 // 128], FP,
                             kind="ExternalInput")
    d_last = nc.dram_tensor("lastT", [2, 128, nl], FP, kind="ExternalOutput")

    CH = min(512, nl)
    NCH = (nl + CH - 1) // CH

    with tile.TileContext(nc) as tc:
        with ExitStack() as c1:
            cw = c1.enter_context(tc.tile_pool(name="cweights", bufs=1))
            cst = c1.enter_context(tc.tile_pool(name="cstate", bufs=1))
            ein = c1.enter_context(tc.tile_pool(name="ein", bufs=2))
            ctmp = c1.enter_context(tc.tile_pool(name="ctmp", bufs=2))
            cps = c1.enter_context(tc.tile_pool(name="cpsum", bufs=8,
                                                space="PSUM"))
            cWx = cw.tile([E, G4], FR, tag="cWx", name="cWx")
            cWh = cw.tile([128, 2, G4], FR, tag="cWh", name="cWh")
            cb = cw.tile([128, G4 // 128], FP, tag="cb", name="cb")
            lenr = cw.tile([128, nl], FP, tag="lenr", name="lenr")
            nc.sync.dma_start(cWx[:], d_cWxT.ap()[:])
            nc.sync.dma_start(cWh[:], d_cWhT.ap().rearrange("k p g -> p k g"))
            nc.sync.dma_start(cb[:], d_cbias.ap()[:])
            nc.sync.dma_start(lenr[:], d_lenrep.ap()[:])

            last = [cst.tile([128, nl], FP, tag=f"last{j}", name=f"last{j}")
                    for j in range(2)]
            hh = [[cst.tile([128, nl], FR, tag=f"h{p}{j}", name=f"h{p}{j}")
                   for j in range(2)] for p in range(2)]
            cc = [cst.tile([128, nl], FP, tag=f"c{j}", name=f"c{j}")
                  for j in range(2)]
            for j in range(2):
                nc.vector.memset(cc[j][:], 0.0)
                nc.vector.memset(last[j][:], 0.0)

            for t in range(Lc):
                et = ein.tile([E, nl], FR, tag="et", name="et")
                nc.sync.dma_start(et[:], d_eT.ap()[t])
                hprev = hh[t % 2]
                hcur = hh[(t + 1) % 2]
                for ci in range(NCH):
                    cs = slice(ci * CH, (ci + 1) * CH)
                    mask = ctmp.tile([128, CH], FP, tag="mask", name="mask")
                    nc.gpsimd.tensor_scalar(mask[:], lenr[:, cs], float(t),
                                            None, op0=mybir.AluOpType.is_gt)
                    for j in range(2):
                        ps = {}
                        for gi in range(4):
                            m = 2 * gi + j
                            p = cps.tile([128, CH], FP, tag="ps", name="ps")
                            ps[(j, gi)] = p
                            mm = [(cWx[:, m * 128:(m + 1) * 128], et[:, cs])]
                            if t > 0:
                                for k in range(2):
                                    mm.append((cWh[:, k, m * 128:(m + 1) * 128],
                                               hprev[k][:, cs]))
                            for ki, (lhsT, rhs) in enumerate(mm):
                                nc.tensor.matmul(p[:], lhsT, rhs,
                                                 start=(ki == 0),
                                                 stop=(ki == len(mm) - 1))
                        bias = [cb[:, (2 * gi + j):(2 * gi + j) + 1]
                                for gi in range(4)]
                        i_s = ctmp.tile([128, CH], FP, tag="i_s", name="i_s")
                        f_s = ctmp.tile([128, CH], FP, tag="f_s", name="f_s")
                        g_t = ctmp.tile([128, CH], FP, tag="g_t", name="g_t")
                        o_s = ctmp.tile([128, CH], FP, tag="o_s", name="o_s")
                        nc.scalar.activation(i_s[:], ps[(j, 0)][:], Sig,
                                             bias=bias[0])
                        nc.scalar.activation(f_s[:], ps[(j, 1)][:], Sig,
                                             bias=bias[1])
                        nc.scalar.activation(g_t[:], ps[(j, 2)][:], TanhF,
                                             bias=bias[2])
                        nc.scalar.activation(o_s[:], ps[(j, 3)][:], Sig,
                                             bias=bias[3])
                        ig = ctmp.tile([128, CH], FP, tag="ig", name="ig")
                        nc.vector.tensor_mul(ig[:], i_s[:], g_t[:])
                        nc.gpsimd.tensor_mul(cc[j][:, cs], f_s[:], cc[j][:, cs])
                        nc.vector.tensor_add(cc[j][:, cs], cc[j][:, cs], ig[:])
                        tc_t = ctmp.tile([128, CH], FP, tag="tc", name="tc")
                        nc.scalar.activation(tc_t[:], cc[j][:, cs], TanhF)
                        nc.vector.tensor_mul(hcur[j][:, cs], o_s[:], tc_t[:])
                        dd = ctmp.tile([128, CH], FP, tag="dd", name="dd")
                        nc.gpsimd.tensor_sub(dd[:], hcur[j][:, cs],
                                             last[j][:, cs])
                        nc.vector.tensor_mul(dd[:], dd[:], mask[:])
                        nc.vector.tensor_add(last[j][:, cs], last[j][:, cs],
                                             dd[:])
            for j in range(2):
                nc.sync.dma_start(d_last.ap()[j], last[j][:])
    nc.compile()
    return nc


def build_l2(bl2=2 * BL):
    """Word LSTM, one direction per core over bl2 sentences."""
    nl = bl2 * S
    nc = bacc.Bacc("TRN2", target_bir_lowering=False, debug=False,
                   num_devices=NCORE)
    d_last = nc.dram_tensor("lastT2", [2, 128, nl], FR, kind="ExternalInput")
    d_wW = nc.dram_tensor("wW", [6, 128, WG], FR, kind="ExternalInput")
    d_wb = nc.dram_tensor("wb", [1, WG], FR, kind="ExternalInput")
    d_ones = nc.dram_tensor("onesr", [1, 128], FR, kind="ExternalInput")
    d_eye = nc.dram_tensor("eye", [128, 128], FP, kind="ExternalInput")
    d_hs = nc.dram_tensor("hsTh", [4, 128, nl], BF, kind="ExternalOutput")

    with tile.TileContext(nc) as tc:
        with ExitStack() as c2:
            ww = c2.enter_context(tc.tile_pool(name="wweights", bufs=1))
            wst = c2.enter_context(tc.tile_pool(name="wstate", bufs=1))
            wtmp = c2.enter_context(tc.tile_pool(name="wtmp", bufs=2))
            wps = c2.enter_context(tc.tile_pool(name="wpsum", bufs=4,
                                                space="PSUM"))
            eye_sb = ww.tile([128, 128], FP, tag="eye", name="eye")
            nc.sync.dma_start(eye_sb[:], d_eye.ap()[:])
            ones = ww.tile([1, bl2], FR, tag="ones", name="ones")
            nc.sync.dma_start(ones[:], d_ones.ap()[:, 0:bl2])
            wbt = ww.tile([1, WG], FR, tag="wbt", name="wbt")
            nc.sync.dma_start(wbt[:], d_wb.ap()[:])
            wsb = ww.tile([128, 6, WG], FR, tag="wsb", name="wsb")
            nc.sync.dma_start(wsb[:], d_wW.ap().rearrange("k p g -> p k g"))

            lastT = [ww.tile([128, nl], FR, tag=f"lastT{j}", name=f"lastT{j}")
                     for j in range(2)]
            for j in range(2):
                nc.sync.dma_start(lastT[j][:], d_last.ap()[j])
            hsT = [wst.tile([128, nl], BF, tag=f"hsT{k}", name=f"hsT{k}")
                   for k in range(4)]
            cstate = wst.tile([bl2, H], FP, tag="wc", name="wc")
            nc.vector.memset(cstate[:], 0.0)
            ring = wst.tile([128, 4 * bl2], FR, tag="ring", name="ring")
            lastv = [lastT[j].rearrange("p (b s) -> p s b", s=S)
                     for j in range(2)]
            hsTv = [hsT[k].rearrange("p (b s) -> p s b", s=S)
                    for k in range(4)]

            # x-part + bias of step s are issued one iteration early
            def xb_mms(s, gp):
                for gc in range(4):
                    ns = slice(gc * H, (gc + 1) * H)
                    nc.tensor.matmul(gp[gc][:], ones[:], wbt[:, ns],
                                     start=True, stop=False)
                    for k in range(2):
                        nc.tensor.matmul(gp[gc][:], lastv[k][:, s, :],
                                         wsb[:, k, ns],
                                         start=False, stop=(s == 0 and k == 1))

            gps = [wps.tile([bl2, H], FP, tag="wps", name="wps", bufs=6)
                   for _ in range(4)]
            xb_mms(0, gps)
            for s in range(S):
                if s > 0:
                    for gc in range(4):
                        ns = slice(gc * H, (gc + 1) * H)
                        for k in range(4):
                            nc.tensor.matmul(
                                gps[gc][:],
                                ring[:, k * bl2:(k + 1) * bl2],
                                wsb[:, 2 + k, ns],
                                start=False, stop=(k == 3))
                i_s = wtmp.tile([bl2, H], FP, tag="wi", name="wi")
                f_s = wtmp.tile([bl2, H], FP, tag="wf", name="wf")
                g_t = wtmp.tile([bl2, H], FP, tag="wg", name="wg")
                o_s = wtmp.tile([bl2, H], FP, tag="wo", name="wo")
                nc.scalar.activation(i_s[:], gps[0][:], Sig)
                nc.scalar.activation(f_s[:], gps[1][:], Sig)
                nc.scalar.activation(g_t[:], gps[2][:], TanhF)
                nc.scalar.activation(o_s[:], gps[3][:], Sig)
                ig = wtmp.tile([bl2, H], FP, tag="wig", name="wig")
                tc_t = wtmp.tile([bl2, H], FP, tag="wtc", name="wtc")
                hrow = wtmp.tile([bl2, H], FP, tag="whr", name="whr")
                if s + 1 < S:
                    gps = [wps.tile([bl2, H], FP, tag="wps", name="wps",
                                    bufs=6) for _ in range(4)]
                    xb_mms(s + 1, gps)
                tp4 = wps.tile([128, 4 * bl2], FP, tag="tp4", name="tp4",
                               bufs=2)
                # fully chunk-pipelined tail: each 128-wide hidden slice runs
                # cell-update -> tanh -> h -> transpose -> ring independently,
                # so the next step's first recurrent matmul starts early
                for k in range(4):
                    ks = slice(k * 128, (k + 1) * 128)
                    nc.vector.tensor_mul(ig[:, ks], i_s[:, ks], g_t[:, ks])
                    nc.gpsimd.tensor_mul(cstate[:, ks], f_s[:, ks],
                                         cstate[:, ks])
                    nc.vector.tensor_add(cstate[:, ks], cstate[:, ks],
                                         ig[:, ks])
                    nc.scalar.activation(tc_t[:, ks], cstate[:, ks], TanhF)
                    nc.gpsimd.tensor_mul(hrow[:, ks], o_s[:, ks], tc_t[:, ks])
                    nc.tensor.transpose(tp4[:, k * bl2:(k + 1) * bl2],
                                        hrow[:, ks], eye_sb[0:bl2, 0:bl2])
                    nc.vector.tensor_copy(ring[:, k * bl2:(k + 1) * bl2],
                                          tp4[:, k * bl2:(k + 1) * bl2])
                    nc.vector.tensor_copy(hsTv[k][:, s, :],
                                          ring[:, k * bl2:(k + 1) * bl2])
            for k in range(4):
                nc.sync.dma_start(d_hs.ap()[k], hsT[k][:])
    nc.compile()
    return nc


def build_l3(bl=BL):
    """MLP + log_softmax, data-parallel."""
    nl = bl * S
    nc = bacc.Bacc("TRN2", target_bir_lowering=False, debug=False,
                   num_devices=NCORE)
    d_hs = nc.dram_tensor("hsT8", [8, 128, nl], BF, kind="ExternalInput")
    d_W1T = nc.dram_tensor("W1T", [8, 128, 256], BF, kind="ExternalInput")
    d_b1 = nc.dram_tensor("b1m", [128, 2], FP, kind="ExternalInput")
    d_W2T = nc.dram_tensor("W2T", [2, 128, 256], BF, kind="ExternalInput")
    d_b2 = nc.dram_tensor("b2m", [128, 2], FP, kind="ExternalInput")
    d_W3T = nc.dram_tensor("W3T", [2, 128, OUT], BF, kind="ExternalInput")
    d_b3 = nc.dram_tensor("b3m", [OUT, 1], FP, kind="ExternalInput")
    d_eye = nc.dram_tensor("eye", [128, 128], FP, kind="ExternalInput")
    d_y = nc.dram_tensor("y", [nl, OUT], FP, kind="ExternalOutput")

    CH = min(512, nl)
    NCH = (nl + CH - 1) // CH

    with tile.TileContext(nc) as tc:
        with ExitStack() as c3:
            mw = c3.enter_context(tc.tile_pool(name="mweights", bufs=1))
            mact = c3.enter_context(tc.tile_pool(name="mact", bufs=1))
            mtmp = c3.enter_context(tc.tile_pool(name="mtmp", bufs=4))
            mps = c3.enter_context(tc.tile_pool(name="mpsum", bufs=2,
                                                space="PSUM"))
            sps = c3.enter_context(tc.tile_pool(name="spsum", bufs=2,
                                                space="PSUM"))
            eye_sb = mw.tile([128, 128], FP, tag="eye", name="eye")
            nc.sync.dma_start(eye_sb[:], d_eye.ap()[:])
            W1 = mw.tile([128, 8, 256], BF, tag="W1", name="W1")
            W2 = mw.tile([128, 2, 256], BF, tag="W2", name="W2")
            W3 = mw.tile([128, 2, OUT], BF, tag="W3", name="W3")
            b1 = mw.tile([128, 2], FP, tag="b1", name="b1")
            b2 = mw.tile([128, 2], FP, tag="b2", name="b2")
            b3 = mw.tile([OUT, 1], FP, tag="b3", name="b3")
            nc.sync.dma_start(W1[:], d_W1T.ap().rearrange("k p g -> p k g"))
            nc.sync.dma_start(W2[:], d_W2T.ap().rearrange("k p g -> p k g"))
            nc.sync.dma_start(W3[:], d_W3T.ap().rearrange("k p g -> p k g"))
            nc.sync.dma_start(b1[:], d_b1.ap()[:])
            nc.sync.dma_start(b2[:], d_b2.ap()[:])
            nc.sync.dma_start(b3[:], d_b3.ap()[:])
            hsT = [mw.tile([128, nl], BF, tag=f"hsT{k}", name=f"hsT{k}")
                   for k in range(8)]
            for k in range(8):
                nc.sync.dma_start(hsT[k][:], d_hs.ap()[k])
            h1 = [mact.tile([128, nl], BF, tag=f"h1{m}", name=f"h1{m}")
                  for m in range(2)]
            h2 = [mact.tile([128, nl], BF, tag=f"h2{m}", name=f"h2{m}")
                  for m in range(2)]
            for ci in range(NCH):
                cs = slice(ci * CH, (ci + 1) * CH)
                for m in range(2):
                    p = mps.tile([128, CH], FP, tag="mp1", name="mp1")
                    for k in range(8):
                        nc.tensor.matmul(
                            p[:], W1[:, k, m * 128:(m + 1) * 128],
                            hsT[k][:, cs], start=(k == 0), stop=(k == 7))
                    nc.scalar.activation(h1[m][:, cs], p[:], ReluF,
                                         bias=b1[:, m:m + 1])
            for ci in range(NCH):
                cs = slice(ci * CH, (ci + 1) * CH)
                for m in range(2):
                    p = mps.tile([128, CH], FP, tag="mp2", name="mp2")
                    for k in range(2):
                        nc.tensor.matmul(
                            p[:], W2[:, k, m * 128:(m + 1) * 128],
                            h1[k][:, cs], start=(k == 0), stop=(k == 1))
                    nc.scalar.activation(h2[m][:, cs], p[:], ReluF,
                                         bias=b2[:, m:m + 1])
            # two passes batched by ACT function: all Exp, then all Ln,
            # so the Exp/Ln activation tables load once each instead of
            # per position-tile
            npt = max(1, nl // 128)
            lgs = [mact.tile([128, OUT], FP, tag=f"lgs{pi}", name=f"lgs{pi}")
                   for pi in range(npt)]
            nmxs = [mact.tile([128, 1], FP, tag=f"nmx{pi}", name=f"nmx{pi}")
                    for pi in range(npt)]
            sms = [mact.tile([128, 1], FP, tag=f"sm{pi}", name=f"sm{pi}")
                   for pi in range(npt)]
            for pi in range(npt):
                pcount = min(128, nl - pi * 128)
                psl = slice(pi * 128, pi * 128 + pcount)
                lg = mps.tile([OUT, pcount], FP, tag="mp3", name="mp3")
                for k in range(2):
                    nc.tensor.matmul(lg[:], W3[:, k, :], h2[k][:, psl],
                                     start=(k == 0), stop=(k == 1))
                lgb = mtmp.tile([OUT, pcount], FP, tag="lgb", name="lgb")
                nc.scalar.activation(lgb[:], lg[:], IdentF, bias=b3[:, 0:1])
                lgr = sps.tile([pcount, OUT], FP, tag="lgr", name="lgr")
                nc.tensor.transpose(lgr[:], lgb[:], eye_sb[0:OUT, 0:OUT])
                nc.vector.tensor_reduce(nmxs[pi][0:pcount, :], lgr[:],
                                        axis=mybir.AxisListType.X,
                                        op=mybir.AluOpType.max, negate=True)
                ex = mtmp.tile([pcount, OUT], FP, tag="ex", name="ex")
                nc.scalar.activation(ex[:], lgr[:], ExpF,
                                     bias=nmxs[pi][0:pcount, :],
                                     accum_out=sms[pi][0:pcount, :])
                nc.vector.tensor_copy(lgs[pi][0:pcount, :], lgr[:])
            for pi in range(npt):
                pcount = min(128, nl - pi * 128)
                psl = slice(pi * 128, pi * 128 + pcount)
                lsm = mtmp.tile([pcount, 1], FP, tag="lsm", name="lsm")
                nc.scalar.activation(lsm[:], sms[pi][0:pcount, :], LnF)
                shift = mtmp.tile([pcount, 1], FP, tag="shift", name="shift")
                nc.vector.tensor_sub(shift[:], nmxs[pi][0:pcount, :], lsm[:])
                yt = mtmp.tile([pcount, OUT], FP, tag="yt", name="yt")
                nc.vector.tensor_scalar(yt[:], lgs[pi][0:pcount, :],
                                        shift[:], None,
                                        op0=mybir.AluOpType.add)
                nc.sync.dma_start(d_y.ap()[psl, :], yt[:])
    nc.compile()
    return nc


@functools.lru_cache(maxsize=4)
def _modules(bl):
    return build_l1(bl), build_l2(2 * bl), build_l3(bl)


def _prep_shared(inputs):
    f32 = np.float32
    cWxT = np.ascontiguousarray(np.asarray(inputs["cW_ih"], f32).T)
    cWhT = np.ascontiguousarray(
        np.asarray(inputs["cW_hh"], f32).T).reshape(2, 128, G4)
    cbias = (np.asarray(inputs["cb_ih"], f32)
             + np.asarray(inputs["cb_hh"], f32))
    cbias_m = np.ascontiguousarray(cbias.reshape(G4 // 128, 128).T)
    wW, wb = [], []
    for pre in ("f", "b"):
        wih = np.asarray(inputs[pre + "W_ih"], f32)
        whh = np.asarray(inputs[pre + "W_hh"], f32)
        wW.append(np.ascontiguousarray(
            np.concatenate([wih.T, whh.T], 0)).reshape(6, 128, WG))
        wb.append((np.asarray(inputs[pre + "b_ih"], f32)
                   + np.asarray(inputs[pre + "b_hh"], f32)).reshape(1, WG))
    W1T = np.ascontiguousarray(
        np.asarray(inputs["W1"], f32).T.astype(BF_NP)).reshape(8, 128, 256)
    b1m = np.ascontiguousarray(np.asarray(inputs["b1"], f32).reshape(2, 128).T)
    W2T = np.ascontiguousarray(
        np.asarray(inputs["W2"], f32).T.astype(BF_NP)).reshape(2, 128, 256)
    b2m = np.ascontiguousarray(np.asarray(inputs["b2"], f32).reshape(2, 128).T)
    W3T = np.ascontiguousarray(
        np.asarray(inputs["W3"], f32).T.astype(BF_NP)).reshape(2, 128, OUT)
    b3m = np.ascontiguousarray(np.asarray(inputs["b3"], f32).reshape(OUT, 1))
    eye = np.eye(128, dtype=f32)
    onesr = np.ones((1, 128), f32)
    return dict(cWxT=cWxT, cWhT=cWhT, cbias=cbias_m, wW=wW, wb=wb, W1T=W1T,
                b1m=b1m, W2T=W2T, b2m=b2m, W3T=W3T, b3m=b3m, eye=eye,
                onesr=onesr)


def _l1_maps(inputs, sh, bl, ncores):
    x = np.asarray(inputs["x"])
    emb = np.asarray(inputs["emb"], np.float32)
    nl = bl * S
    maps = []
    for c in range(ncores):
        xc = x[c * bl:(c + 1) * bl].reshape(nl, Lc)
        lengths = (xc != 0).sum(axis=1).astype(np.float32)
        lenrep = np.ascontiguousarray(
            np.broadcast_to(lengths[None, :], (128, nl)))
        eT = np.ascontiguousarray(emb[xc].transpose(1, 2, 0))
        maps.append(dict(eT=eT, lenrep=lenrep, cWxT=sh["cWxT"],
                         cWhT=sh["cWhT"], cbias=sh["cbias"]))
    return maps


def _l2_maps(last_full, sh, bl2, ncores):
    # last_full: [2, 128, B*S] f32, columns ordered (b * S + s)
    maps = []
    half = ncores // 2
    for c in range(ncores):
        d = 0 if c < half else 1
        g = c % half
        lo, hi = g * bl2 * S, (g + 1) * bl2 * S
        lt = last_full[:, :, lo:hi]
        if d == 1:
            lt = lt.reshape(2, 128, bl2, S)[:, :, :, ::-1].reshape(
                2, 128, bl2 * S)
        maps.append(dict(lastT2=np.ascontiguousarray(lt), wW=sh["wW"][d],
                         wb=sh["wb"][d], onesr=sh["onesr"], eye=sh["eye"]))
    return maps


def _l3_maps(hs_f, hs_b, sh, bl, ncores):
    # hs_f/hs_b: [4, 128, B*S] bf16, full batch
    nl = bl * S
    maps = []
    for c in range(ncores):
        lo, hi = c * nl, (c + 1) * nl
        hs8 = np.concatenate([hs_f[:, :, lo:hi], hs_b[:, :, lo:hi]], axis=0)
        maps.append(dict(hsT8=np.ascontiguousarray(hs8), W1T=sh["W1T"],
                         b1m=sh["b1m"], W2T=sh["W2T"], b2m=sh["b2m"],
                         W3T=sh["W3T"], b3m=sh["b3m"], eye=sh["eye"]))
    return maps


def _pipeline(inputs, bl, ncores, run_l1, run_l2, run_l3):
    """Shared 3-launch pipeline; run_lX(in_maps) -> list of output dicts."""
    sh = _prep_shared(inputs)
    bl2 = 2 * bl
    half = ncores // 2

    r1 = run_l1(_l1_maps(inputs, sh, bl, ncores))
    last_full = np.concatenate([r1[c]["lastT"] for c in range(ncores)],
                               axis=2)

    r2 = run_l2(_l2_maps(last_full, sh, bl2, ncores))
    hs_f = np.concatenate([r2[g]["hsTh"] for g in range(half)], axis=2)
    hsb_parts = []
    for g in range(half):
        hb = np.asarray(r2[half + g]["hsTh"]).reshape(
            4, 128, bl2, S)[:, :, :, ::-1]
        hsb_parts.append(hb.reshape(4, 128, bl2 * S))
    hs_b = np.concatenate(hsb_parts, axis=2)

    r3 = run_l3(_l3_maps(hs_f, hs_b, sh, bl, ncores))
    out = np.empty((bl * ncores, S, OUT), np.float32)
    for c in range(ncores):
        out[c * bl:(c + 1) * bl] = np.asarray(r3[c]["y"]).reshape(bl, S, OUT)
    return out


def kernel(**inputs):
    l1, l2, l3 = _modules(BL)

    def runner(nc):
        def run(in_maps):
            res = bass_utils.run_bass_kernel_spmd(
                nc, in_maps, core_ids=list(range(NCORE)))
            return res.results
        return run

    return _pipeline(inputs, BL, NCORE, runner(l1), runner(l2), runner(l3))



# revision 25
# speedup vs baseline: 1.1315x; 1.1315x over previous
"""Trainium2 Bass kernel for nn_CharTaggerBiLSTM, 8-core SPMD, 3 launches.

L1 char LSTM: data-parallel over batch (16 sentences/core). Transposed
   layout (features-on-partitions), f32r matmuls; emits the masked last
   hidden state per word -> DRAM.
L2 word LSTM: one direction per core (cores 0-3 forward, 4-7 backward),
   32 sentences/core so each weight stream serves twice the rows.
   Direction is data: backward cores receive the char outputs with the
   sentence axis reversed on host and their outputs are un-reversed.
   x-part/bias matmuls for step s+1 are issued during step s's
   elementwise work to keep PE fed.
L3 MLP + log_softmax: data-parallel (16 sentences/core), bf16 GEMMs.

Host does embedding gather, weight reshapes, the two reshard steps, and
reassembly.
"""

import sys
import functools
from contextlib import ExitStack

sys.path.insert(0, "/opt/trn_rl_repo")

import numpy as np
import ml_dtypes
from concourse import bacc, bass, mybir, tile, bass_utils

BF_NP = ml_dtypes.bfloat16
U8 = mybir.dt.uint8
E = 64


B, S, Lc = 128, 128, 20
AB, E = 100, 64
Hc, H, OUT = 256, 512, 50
NCORE = 8
BL = B // NCORE            # sentences per core in L1/L3
FP = mybir.dt.float32
FR = mybir.dt.float32r
BF = mybir.dt.bfloat16
G4 = 4 * Hc
WG = 4 * H

Sig = mybir.ActivationFunctionType.Sigmoid
TanhF = mybir.ActivationFunctionType.Tanh
ReluF = mybir.ActivationFunctionType.Relu
ExpF = mybir.ActivationFunctionType.Exp
LnF = mybir.ActivationFunctionType.Ln
IdentF = mybir.ActivationFunctionType.Identity


def build_l1(bl=BL, lmin=18):
    """Char LSTM, data-parallel; writes lastT [2,128,nl] bf16 to DRAM."""
    nl = bl * S
    nc = bacc.Bacc("TRN2", target_bir_lowering=False, debug=False,
                   num_devices=NCORE)
    d_eT = nc.dram_tensor("eT", [Lc, E, nl], BF, kind="ExternalInput")
    d_lenrep = nc.dram_tensor("lenrep", [128, nl], BF, kind="ExternalInput")
    d_cWx2 = nc.dram_tensor("cWx2", [128, 4, 128], BF, kind="ExternalInput")
    d_cWhT = nc.dram_tensor("cWhT", [2, 128, G4], BF, kind="ExternalInput")
    d_cbias = nc.dram_tensor("cbias", [128, G4 // 128], FP,
                             kind="ExternalInput")
    d_last = nc.dram_tensor("lastT", [2, 128, nl], BF, kind="ExternalOutput")

    CH = 1024
    NCH = nl // CH
    GF = [Sig, Sig, TanhF, Sig]          # gate funcs for gi = i, f, g, o

    with tile.TileContext(nc) as tc:
        with ExitStack() as c1:
            cw = c1.enter_context(tc.tile_pool(name="cweights", bufs=1))
            cst = c1.enter_context(tc.tile_pool(name="cstate", bufs=1))
            ein = c1.enter_context(tc.tile_pool(name="ein", bufs=2))
            ctmp = c1.enter_context(tc.tile_pool(name="ctmp", bufs=3))
            cps = c1.enter_context(tc.tile_pool(name="cpsum", bufs=4,
                                                space="PSUM"))
            cWx2 = cw.tile([128, 4, 128], BF, tag="cWx2", name="cWx2")
            cWh = cw.tile([128, 2, G4], BF, tag="cWh", name="cWh")
            cb = cw.tile([128, G4 // 128], FP, tag="cb", name="cb")
            lenr = cw.tile([128, nl], BF, tag="lenr", name="lenr")
            nc.sync.dma_start(cWx2[:], d_cWx2.ap()[:])
            nc.sync.dma_start(cWh[:], d_cWhT.ap().rearrange("k p g -> p k g"))
            nc.sync.dma_start(cb[:], d_cbias.ap()[:])
            nc.sync.dma_start(lenr[:], d_lenrep.ap()[:])

            last = cst.tile([128, 2, nl], BF, tag="last", name="last")
            hh = [cst.tile([128, 2, nl], BF, tag=f"h{p}", name=f"h{p}")
                  for p in range(2)]
            cc = cst.tile([128, 2, nl], BF, tag="cc", name="cc")
            nc.vector.memset(cc[:], 0.0)
            nc.vector.memset(last[:], 0.0)

            for t in range(Lc):
                et2 = ein.tile([128, nl], BF, tag="et2", name="et2")
                nc.sync.dma_start(et2[0:E, :], d_eT.ap()[t])
                nc.sync.dma_start(et2[E:128, :], d_eT.ap()[t])
                masked = t >= lmin - 1
                hprev = hh[t % 2]
                hcur = hh[(t + 1) % 2]
                for ci in range(NCH):
                    cs = slice(ci * CH, (ci + 1) * CH)
                    if masked:
                        mk = ctmp.tile([128, CH], U8, tag="mk", name="mk")
                        nc.gpsimd.tensor_scalar(mk[:], lenr[:, cs], float(t),
                                                None,
                                                op0=mybir.AluOpType.is_gt)
                    for j in range(2):
                        ps = [cps.tile([128, CH], FP, tag="ps", name="ps")
                              for _ in range(4)]
                        # x-part: two K=64 row strips per PE pass, plus
                        # bias-free accumulation of the two h chunks;
                        # matmul outputs are split into 512-col halves
                        # (one PSUM bank each)
                        for hw_ in range(2):
                            o5 = slice(hw_ * 512, (hw_ + 1) * 512)
                            c5 = slice(ci * CH + hw_ * 512,
                                       ci * CH + (hw_ + 1) * 512)
                            for pi in range(2):
                                sl = 2 * j + pi
                                nc.tensor.matmul(ps[2 * pi][:, o5],
                                                 cWx2[0:E, sl, :],
                                                 et2[0:E, c5],
                                                 start=True, stop=(t == 0))
                                nc.tensor.matmul(ps[2 * pi + 1][:, o5],
                                                 cWx2[E:128, sl, :],
                                                 et2[E:128, c5],
                                                 start=True, stop=(t == 0))
                            if t > 0:
                                for gi in range(4):
                                    m = 2 * gi + j
                                    for k in range(2):
                                        nc.tensor.matmul(
                                            ps[gi][:, o5],
                                            cWh[:, k, m * 128:(m + 1) * 128],
                                            hprev[:, k, c5],
                                            start=False, stop=(k == 1))
                        gsb = [ctmp.tile([128, CH], BF, tag=f"g{gi}",
                                         name=f"g{gi}") for gi in range(4)]
                        for gi in range(4):
                            m = 2 * gi + j
                            nc.scalar.activation(gsb[gi][:], ps[gi][:],
                                                 GF[gi], bias=cb[:, m:m + 1])
                        ig = ctmp.tile([128, CH], BF, tag="ig", name="ig")
                        nc.vector.tensor_mul(ig[:], gsb[0][:], gsb[2][:])
                        fc = ctmp.tile([128, CH], BF, tag="fc", name="fc")
                        nc.vector.tensor_mul(fc[:], gsb[1][:], cc[:, j, cs])
                        nc.vector.tensor_add(cc[:, j, cs], fc[:], ig[:])
                        tct = ctmp.tile([128, CH], BF, tag="tct", name="tct")
                        nc.scalar.activation(tct[:], cc[:, j, cs], TanhF)
                        nc.vector.tensor_mul(hcur[:, j, cs], gsb[3][:],
                                             tct[:])
                        if masked:
                            nc.vector.copy_predicated(last[:, j, cs], mk[:],
                                                      hcur[:, j, cs])
            for j in range(2):
                nc.sync.dma_start(d_last.ap()[j], last[:, j, :])
    nc.compile()
    return nc


def build_l2(bl2=2 * BL):
    """Word LSTM, one direction per core over bl2 sentences."""
    nl = bl2 * S
    nc = bacc.Bacc("TRN2", target_bir_lowering=False, debug=False,
                   num_devices=NCORE)
    d_last = nc.dram_tensor("lastT2", [2, 128, nl], FR, kind="ExternalInput")
    d_wW = nc.dram_tensor("wW", [6, 128, WG], FR, kind="ExternalInput")
    d_wb = nc.dram_tensor("wb", [1, WG], FR, kind="ExternalInput")
    d_ones = nc.dram_tensor("onesr", [1, 128], FR, kind="ExternalInput")
    d_eye = nc.dram_tensor("eye", [128, 128], FP, kind="ExternalInput")
    d_hs = nc.dram_tensor("hsTh", [4, 128, nl], BF, kind="ExternalOutput")

    with tile.TileContext(nc) as tc:
        with ExitStack() as c2:
            ww = c2.enter_context(tc.tile_pool(name="wweights", bufs=1))
            wst = c2.enter_context(tc.tile_pool(name="wstate", bufs=1))
            wtmp = c2.enter_context(tc.tile_pool(name="wtmp", bufs=2))
            wps = c2.enter_context(tc.tile_pool(name="wpsum", bufs=4,
                                                space="PSUM"))
            eye_sb = ww.tile([128, 128], FP, tag="eye", name="eye")
            nc.sync.dma_start(eye_sb[:], d_eye.ap()[:])
            ones = ww.tile([1, bl2], FR, tag="ones", name="ones")
            nc.sync.dma_start(ones[:], d_ones.ap()[:, 0:bl2])
            wbt = ww.tile([1, WG], FR, tag="wbt", name="wbt")
            nc.sync.dma_start(wbt[:], d_wb.ap()[:])
            wsb = ww.tile([128, 6, WG], FR, tag="wsb", name="wsb")
            nc.sync.dma_start(wsb[:], d_wW.ap().rearrange("k p g -> p k g"))

            lastT = [ww.tile([128, nl], FR, tag=f"lastT{j}", name=f"lastT{j}")
                     for j in range(2)]
            for j in range(2):
                nc.sync.dma_start(lastT[j][:], d_last.ap()[j])
            hsT = [wst.tile([128, nl], BF, tag=f"hsT{k}", name=f"hsT{k}")
                   for k in range(4)]
            cstate = wst.tile([bl2, H], FP, tag="wc", name="wc")
            nc.vector.memset(cstate[:], 0.0)
            ring = wst.tile([128, 4 * bl2], FR, tag="ring", name="ring")
            lastv = [lastT[j].rearrange("p (b s) -> p s b", s=S)
                     for j in range(2)]
            hsTv = [hsT[k].rearrange("p (b s) -> p s b", s=S)
                    for k in range(4)]

            # x-part + bias of step s are issued one iteration early
            def xb_mms(s, gp):
                for gc in range(4):
                    ns = slice(gc * H, (gc + 1) * H)
                    nc.tensor.matmul(gp[gc][:], ones[:], wbt[:, ns],
                                     start=True, stop=False)
                    for k in range(2):
                        nc.tensor.matmul(gp[gc][:], lastv[k][:, s, :],
                                         wsb[:, k, ns],
                                         start=False, stop=(s == 0 and k == 1))

            gps = [wps.tile([bl2, H], FP, tag="wps", name="wps", bufs=6)
                   for _ in range(4)]
            xb_mms(0, gps)
            for s in range(S):
                if s > 0:
                    for gc in range(4):
                        ns = slice(gc * H, (gc + 1) * H)
                        for k in range(4):
                            nc.tensor.matmul(
                                gps[gc][:],
                                ring[:, k * bl2:(k + 1) * bl2],
                                wsb[:, 2 + k, ns],
                                start=False, stop=(k == 3))
                i_s = wtmp.tile([bl2, H], FP, tag="wi", name="wi")
                f_s = wtmp.tile([bl2, H], FP, tag="wf", name="wf")
                g_t = wtmp.tile([bl2, H], FP, tag="wg", name="wg")
                o_s = wtmp.tile([bl2, H], FP, tag="wo", name="wo")
                nc.scalar.activation(i_s[:], gps[0][:], Sig)
                nc.scalar.activation(f_s[:], gps[1][:], Sig)
                nc.scalar.activation(g_t[:], gps[2][:], TanhF)
                nc.scalar.activation(o_s[:], gps[3][:], Sig)
                ig = wtmp.tile([bl2, H], FP, tag="wig", name="wig")
                tc_t = wtmp.tile([bl2, H], FP, tag="wtc", name="wtc")
                hrow = wtmp.tile([bl2, H], FP, tag="whr", name="whr")
                if s + 1 < S:
                    gps = [wps.tile([bl2, H], FP, tag="wps", name="wps",
                                    bufs=6) for _ in range(4)]
                    xb_mms(s + 1, gps)
                tp4 = wps.tile([128, 4 * bl2], FP, tag="tp4", name="tp4",
                               bufs=2)
                # fully chunk-pipelined tail: each 128-wide hidden slice runs
                # cell-update -> tanh -> h -> transpose -> ring independently,
                # so the next step's first recurrent matmul starts early
                for k in range(4):
                    ks = slice(k * 128, (k + 1) * 128)
                    nc.vector.tensor_mul(ig[:, ks], i_s[:, ks], g_t[:, ks])
                    nc.gpsimd.tensor_mul(cstate[:, ks], f_s[:, ks],
                                         cstate[:, ks])
                    nc.vector.tensor_add(cstate[:, ks], cstate[:, ks],
                                         ig[:, ks])
                    nc.scalar.activation(tc_t[:, ks], cstate[:, ks], TanhF)
                    nc.gpsimd.tensor_mul(hrow[:, ks], o_s[:, ks], tc_t[:, ks])
                    nc.tensor.transpose(tp4[:, k * bl2:(k + 1) * bl2],
                                        hrow[:, ks], eye_sb[0:bl2, 0:bl2])
                    nc.vector.tensor_copy(ring[:, k * bl2:(k + 1) * bl2],
                                          tp4[:, k * bl2:(k + 1) * bl2])
                    nc.vector.tensor_copy(hsTv[k][:, s, :],
                                          ring[:, k * bl2:(k + 1) * bl2])
            for k in range(4):
                nc.sync.dma_start(d_hs.ap()[k], hsT[k][:])
    nc.compile()
    return nc


def build_l3(bl=BL):
    """MLP + log_softmax, data-parallel."""
    nl = bl * S
    nc = bacc.Bacc("TRN2", target_bir_lowering=False, debug=False,
                   num_devices=NCORE)
    d_hs = nc.dram_tensor("hsT8", [8, 128, nl], BF, kind="ExternalInput")
    d_W1T = nc.dram_tensor("W1T", [8, 128, 256], BF, kind="ExternalInput")
    d_b1 = nc.dram_tensor("b1m", [128, 2], FP, kind="ExternalInput")
    d_W2T = nc.dram_tensor("W2T", [2, 128, 256], BF, kind="ExternalInput")
    d_b2 = nc.dram_tensor("b2m", [128, 2], FP, kind="ExternalInput")
    d_W3T = nc.dram_tensor("W3T", [2, 128, OUT], BF, kind="ExternalInput")
    d_b3 = nc.dram_tensor("b3m", [OUT, 1], FP, kind="ExternalInput")
    d_eye = nc.dram_tensor("eye", [128, 128], FP, kind="ExternalInput")
    d_y = nc.dram_tensor("y", [nl, OUT], FP, kind="ExternalOutput")

    CH = min(512, nl)
    NCH = (nl + CH - 1) // CH

    with tile.TileContext(nc) as tc:
        with ExitStack() as c3:
            mw = c3.enter_context(tc.tile_pool(name="mweights", bufs=1))
            mact = c3.enter_context(tc.tile_pool(name="mact", bufs=1))
            mtmp = c3.enter_context(tc.tile_pool(name="mtmp", bufs=4))
            mps = c3.enter_context(tc.tile_pool(name="mpsum", bufs=2,
                                                space="PSUM"))
            sps = c3.enter_context(tc.tile_pool(name="spsum", bufs=2,
                                                space="PSUM"))
            eye_sb = mw.tile([128, 128], FP, tag="eye", name="eye")
            nc.sync.dma_start(eye_sb[:], d_eye.ap()[:])
            W1 = mw.tile([128, 8, 256], BF, tag="W1", name="W1")
            W2 = mw.tile([128, 2, 256], BF, tag="W2", name="W2")
            W3 = mw.tile([128, 2, OUT], BF, tag="W3", name="W3")
            b1 = mw.tile([128, 2], FP, tag="b1", name="b1")
            b2 = mw.tile([128, 2], FP, tag="b2", name="b2")
            b3 = mw.tile([OUT, 1], FP, tag="b3", name="b3")
            nc.sync.dma_start(W1[:], d_W1T.ap().rearrange("k p g -> p k g"))
            nc.sync.dma_start(W2[:], d_W2T.ap().rearrange("k p g -> p k g"))
            nc.sync.dma_start(W3[:], d_W3T.ap().rearrange("k p g -> p k g"))
            nc.sync.dma_start(b1[:], d_b1.ap()[:])
            nc.sync.dma_start(b2[:], d_b2.ap()[:])
            nc.sync.dma_start(b3[:], d_b3.ap()[:])
            hsT = [mw.tile([128, nl], BF, tag=f"hsT{k}", name=f"hsT{k}")
                   for k in range(8)]
            for k in range(8):
                nc.sync.dma_start(hsT[k][:], d_hs.ap()[k])
            h1 = [mact.tile([128, nl], BF, tag=f"h1{m}", name=f"h1{m}")
                  for m in range(2)]
            h2 = [mact.tile([128, nl], BF, tag=f"h2{m}", name=f"h2{m}")
                  for m in range(2)]
            for ci in range(NCH):
                cs = slice(ci * CH, (ci + 1) * CH)
                for m in range(2):
                    p = mps.tile([128, CH], FP, tag="mp1", name="mp1")
                    for k in range(8):
                        nc.tensor.matmul(
                            p[:], W1[:, k, m * 128:(m + 1) * 128],
                            hsT[k][:, cs], start=(k == 0), stop=(k == 7))
                    nc.scalar.activation(h1[m][:, cs], p[:], ReluF,
                                         bias=b1[:, m:m + 1])
            for ci in range(NCH):
                cs = slice(ci * CH, (ci + 1) * CH)
                for m in range(2):
                    p = mps.tile([128, CH], FP, tag="mp2", name="mp2")
                    for k in range(2):
                        nc.tensor.matmul(
                            p[:], W2[:, k, m * 128:(m + 1) * 128],
                            h1[k][:, cs], start=(k == 0), stop=(k == 1))
                    nc.scalar.activation(h2[m][:, cs], p[:], ReluF,
                                         bias=b2[:, m:m + 1])
            # two passes batched by ACT function: all Exp, then all Ln,
            # so the Exp/Ln activation tables load once each instead of
            # per position-tile
            npt = max(1, nl // 128)
            lgs = [mact.tile([128, OUT], FP, tag=f"lgs{pi}", name=f"lgs{pi}")
                   for pi in range(npt)]
            nmxs = [mact.tile([128, 1], FP, tag=f"nmx{pi}", name=f"nmx{pi}")
                    for pi in range(npt)]
            sms = [mact.tile([128, 1], FP, tag=f"sm{pi}", name=f"sm{pi}")
                   for pi in range(npt)]
            for pi in range(npt):
                pcount = min(128, nl - pi * 128)
                psl = slice(pi * 128, pi * 128 + pcount)
                lg = mps.tile([OUT, pcount], FP, tag="mp3", name="mp3")
                for k in range(2):
                    nc.tensor.matmul(lg[:], W3[:, k, :], h2[k][:, psl],
                                     start=(k == 0), stop=(k == 1))
                lgb = mtmp.tile([OUT, pcount], FP, tag="lgb", name="lgb")
                nc.scalar.activation(lgb[:], lg[:], IdentF, bias=b3[:, 0:1])
                lgr = sps.tile([pcount, OUT], FP, tag="lgr", name="lgr")
                nc.tensor.transpose(lgr[:], lgb[:], eye_sb[0:OUT, 0:OUT])
                nc.vector.tensor_reduce(nmxs[pi][0:pcount, :], lgr[:],
                                        axis=mybir.AxisListType.X,
                                        op=mybir.AluOpType.max, negate=True)
                ex = mtmp.tile([pcount, OUT], FP, tag="ex", name="ex")
                nc.scalar.activation(ex[:], lgr[:], ExpF,
                                     bias=nmxs[pi][0:pcount, :],
                                     accum_out=sms[pi][0:pcount, :])
                nc.vector.tensor_copy(lgs[pi][0:pcount, :], lgr[:])
            for pi in range(npt):
                pcount = min(128, nl - pi * 128)
                psl = slice(pi * 128, pi * 128 + pcount)
                lsm = mtmp.tile([pcount, 1], FP, tag="lsm", name="lsm")
                nc.scalar.activation(lsm[:], sms[pi][0:pcount, :], LnF)
                shift = mtmp.tile([pcount, 1], FP, tag="shift", name="shift")
                nc.vector.tensor_sub(shift[:], nmxs[pi][0:pcount, :], lsm[:])
                yt = mtmp.tile([pcount, OUT], FP, tag="yt", name="yt")
                nc.vector.tensor_scalar(yt[:], lgs[pi][0:pcount, :],
                                        shift[:], None,
                                        op0=mybir.AluOpType.add)
                nc.sync.dma_start(d_y.ap()[psl, :], yt[:])
    nc.compile()
    return nc


@functools.lru_cache(maxsize=4)
def _modules(bl, lmin=18):
    return build_l1(bl, lmin), build_l2(2 * bl), build_l3(bl)


def _prep_shared(inputs):
    f32 = np.float32
    cWxT = np.asarray(inputs["cW_ih"], f32).T
    cWx2 = np.zeros((128, 4, 128), f32)
    for j in range(2):
        for pi in range(2):
            cWx2[0:E, 2 * j + pi] = cWxT[:, (j + 4 * pi) * 128:
                                         (j + 4 * pi) * 128 + 128]
            cWx2[E:128, 2 * j + pi] = cWxT[:, (2 + j + 4 * pi) * 128:
                                           (2 + j + 4 * pi) * 128 + 128]
    cWhT = np.ascontiguousarray(
        np.asarray(inputs["cW_hh"], f32).T).reshape(2, 128, G4)
    cbias = (np.asarray(inputs["cb_ih"], f32)
             + np.asarray(inputs["cb_hh"], f32))
    cbias_m = np.ascontiguousarray(cbias.reshape(G4 // 128, 128).T)
    wW, wb = [], []
    for pre in ("f", "b"):
        wih = np.asarray(inputs[pre + "W_ih"], f32)
        whh = np.asarray(inputs[pre + "W_hh"], f32)
        wW.append(np.ascontiguousarray(
            np.concatenate([wih.T, whh.T], 0)).reshape(6, 128, WG))
        wb.append((np.asarray(inputs[pre + "b_ih"], f32)
                   + np.asarray(inputs[pre + "b_hh"], f32)).reshape(1, WG))
    W1T = np.ascontiguousarray(
        np.asarray(inputs["W1"], f32).T.astype(BF_NP)).reshape(8, 128, 256)
    b1m = np.ascontiguousarray(np.asarray(inputs["b1"], f32).reshape(2, 128).T)
    W2T = np.ascontiguousarray(
        np.asarray(inputs["W2"], f32).T.astype(BF_NP)).reshape(2, 128, 256)
    b2m = np.ascontiguousarray(np.asarray(inputs["b2"], f32).reshape(2, 128).T)
    W3T = np.ascontiguousarray(
        np.asarray(inputs["W3"], f32).T.astype(BF_NP)).reshape(2, 128, OUT)
    b3m = np.ascontiguousarray(np.asarray(inputs["b3"], f32).reshape(OUT, 1))
    eye = np.eye(128, dtype=f32)
    onesr = np.ones((1, 128), f32)
    return dict(cWx2=cWx2.astype(BF_NP), cWhT=cWhT.astype(BF_NP),
                cbias=cbias_m, wW=wW, wb=wb, W1T=W1T,
                b1m=b1m, W2T=W2T, b2m=b2m, W3T=W3T, b3m=b3m, eye=eye,
                onesr=onesr)


def _l1_maps(inputs, sh, bl, ncores):
    x = np.asarray(inputs["x"])
    emb = np.asarray(inputs["emb"], np.float32).astype(BF_NP)
    nl = bl * S
    maps = []
    for c in range(ncores):
        xc = x[c * bl:(c + 1) * bl].reshape(nl, Lc)
        lengths = (xc != 0).sum(axis=1).astype(np.float32)
        lenrep = np.ascontiguousarray(
            np.broadcast_to(lengths[None, :].astype(BF_NP), (128, nl)))
        eT = np.ascontiguousarray(emb[xc].transpose(1, 2, 0))
        maps.append(dict(eT=eT, lenrep=lenrep, cWx2=sh["cWx2"],
                         cWhT=sh["cWhT"], cbias=sh["cbias"]))
    return maps


def _l2_maps(last_full, sh, bl2, ncores):
    # last_full: [2, 128, B*S] f32, columns ordered (b * S + s)
    maps = []
    half = ncores // 2
    for c in range(ncores):
        d = 0 if c < half else 1
        g = c % half
        lo, hi = g * bl2 * S, (g + 1) * bl2 * S
        lt = last_full[:, :, lo:hi]
        if d == 1:
            lt = lt.reshape(2, 128, bl2, S)[:, :, :, ::-1].reshape(
                2, 128, bl2 * S)
        maps.append(dict(lastT2=np.ascontiguousarray(lt), wW=sh["wW"][d],
                         wb=sh["wb"][d], onesr=sh["onesr"], eye=sh["eye"]))
    return maps


def _l3_maps(hs_f, hs_b, sh, bl, ncores):
    # hs_f/hs_b: [4, 128, B*S] bf16, full batch
    nl = bl * S
    maps = []
    for c in range(ncores):
        lo, hi = c * nl, (c + 1) * nl
        hs8 = np.concatenate([hs_f[:, :, lo:hi], hs_b[:, :, lo:hi]], axis=0)
        maps.append(dict(hsT8=np.ascontiguousarray(hs8), W1T=sh["W1T"],
                         b1m=sh["b1m"], W2T=sh["W2T"], b2m=sh["b2m"],
                         W3T=sh["W3T"], b3m=sh["b3m"], eye=sh["eye"]))
    return maps


def _pipeline(inputs, bl, ncores, run_l1, run_l2, run_l3):
    """Shared 3-launch pipeline; run_lX(in_maps) -> list of output dicts."""
    sh = _prep_shared(inputs)
    bl2 = 2 * bl
    half = ncores // 2

    r1 = run_l1(_l1_maps(inputs, sh, bl, ncores))
    last_full = np.concatenate(
        [np.asarray(r1[c]["lastT"]).astype(np.float32)
         for c in range(ncores)], axis=2)

    r2 = run_l2(_l2_maps(last_full, sh, bl2, ncores))
    hs_f = np.concatenate([r2[g]["hsTh"] for g in range(half)], axis=2)
    hsb_parts = []
    for g in range(half):
        hb = np.asarray(r2[half + g]["hsTh"]).reshape(
            4, 128, bl2, S)[:, :, :, ::-1]
        hsb_parts.append(hb.reshape(4, 128, bl2 * S))
    hs_b = np.concatenate(hsb_parts, axis=2)

    r3 = run_l3(_l3_maps(hs_f, hs_b, sh, bl, ncores))
    out = np.empty((bl * ncores, S, OUT), np.float32)
    for c in range(ncores):
        out[c * bl:(c + 1) * bl] = np.asarray(r3[c]["y"]).reshape(bl, S, OUT)
    return out


def kernel(**inputs):
    x = np.asarray(inputs["x"])
    lmin = int((x.reshape(-1, Lc) != 0).sum(axis=1).min())
    l1, l2, l3 = _modules(BL, lmin)

    def runner(nc):
        def run(in_maps):
            res = bass_utils.run_bass_kernel_spmd(
                nc, in_maps, core_ids=list(range(NCORE)))
            return res.results
        return run

    return _pipeline(inputs, BL, NCORE, runner(l1), runner(l2), runner(l3))



# revision 26
# speedup vs baseline: 1.3914x; 1.2297x over previous
"""Trainium2 Bass kernel for nn_CharTaggerBiLSTM, 8-core SPMD, 3 launches.

L1 char LSTM: data-parallel over batch (16 sentences/core). Transposed
   layout (features-on-partitions), f32r matmuls; emits the masked last
   hidden state per word -> DRAM.
L2 word LSTM: one direction per core (cores 0-3 forward, 4-7 backward),
   32 sentences/core so each weight stream serves twice the rows.
   Direction is data: backward cores receive the char outputs with the
   sentence axis reversed on host and their outputs are un-reversed.
   x-part/bias matmuls for step s+1 are issued during step s's
   elementwise work to keep PE fed.
L3 MLP + log_softmax: data-parallel (16 sentences/core), bf16 GEMMs.

Host does embedding gather, weight reshapes, the two reshard steps, and
reassembly.
"""

import sys
import functools
from contextlib import ExitStack

sys.path.insert(0, "/opt/trn_rl_repo")

import numpy as np
import ml_dtypes
from concourse import bacc, bass, mybir, tile, bass_utils

BF_NP = ml_dtypes.bfloat16
U8 = mybir.dt.uint8
F8T = mybir.dt.float8e4
DR = mybir.MatmulPerfMode.DoubleRow
E = 64


B, S, Lc = 128, 128, 20
AB, E = 100, 64
Hc, H, OUT = 256, 512, 50
NCORE = 8
BL = B // NCORE            # sentences per core in L1/L3
FP = mybir.dt.float32
FR = mybir.dt.float32r
BF = mybir.dt.bfloat16
G4 = 4 * Hc
WG = 4 * H

Sig = mybir.ActivationFunctionType.Sigmoid
TanhF = mybir.ActivationFunctionType.Tanh
ReluF = mybir.ActivationFunctionType.Relu
ExpF = mybir.ActivationFunctionType.Exp
LnF = mybir.ActivationFunctionType.Ln
IdentF = mybir.ActivationFunctionType.Identity


def build_l1(bl=BL, lmin=18):
    """Char LSTM, data-parallel; writes lastT [2,128,nl] bf16 to DRAM."""
    nl = bl * S
    nc = bacc.Bacc("TRN2", target_bir_lowering=False, debug=False,
                   num_devices=NCORE)
    d_eT = nc.dram_tensor("eT", [Lc, E, nl], BF, kind="ExternalInput")
    d_lenrep = nc.dram_tensor("lenrep", [128, nl], BF, kind="ExternalInput")
    d_cWx2 = nc.dram_tensor("cWx2", [128, 4, 128], BF, kind="ExternalInput")
    d_cWhT = nc.dram_tensor("cWhT", [2, 128, G4], BF, kind="ExternalInput")
    d_cbias = nc.dram_tensor("cbias", [128, G4 // 128], FP,
                             kind="ExternalInput")
    d_last = nc.dram_tensor("lastT", [2, 128, nl], BF, kind="ExternalOutput")

    CH = 1024
    NCH = nl // CH
    GF = [Sig, Sig, TanhF, Sig]          # gate funcs for gi = i, f, g, o

    with tile.TileContext(nc) as tc:
        with ExitStack() as c1:
            cw = c1.enter_context(tc.tile_pool(name="cweights", bufs=1))
            cst = c1.enter_context(tc.tile_pool(name="cstate", bufs=1))
            ein = c1.enter_context(tc.tile_pool(name="ein", bufs=2))
            ctmp = c1.enter_context(tc.tile_pool(name="ctmp", bufs=3))
            cps = c1.enter_context(tc.tile_pool(name="cpsum", bufs=4,
                                                space="PSUM"))
            cWx2 = cw.tile([128, 4, 128], BF, tag="cWx2", name="cWx2")
            cWh = cw.tile([128, 2, G4], BF, tag="cWh", name="cWh")
            cb = cw.tile([128, G4 // 128], FP, tag="cb", name="cb")
            lenr = cw.tile([128, nl], BF, tag="lenr", name="lenr")
            nc.sync.dma_start(cWx2[:], d_cWx2.ap()[:])
            nc.sync.dma_start(cWh[:], d_cWhT.ap().rearrange("k p g -> p k g"))
            nc.sync.dma_start(cb[:], d_cbias.ap()[:])
            nc.sync.dma_start(lenr[:], d_lenrep.ap()[:])

            last = cst.tile([128, 2, nl], BF, tag="last", name="last")
            hh = [cst.tile([128, 2, nl], BF, tag=f"h{p}", name=f"h{p}")
                  for p in range(2)]
            cc = cst.tile([128, 2, nl], BF, tag="cc", name="cc")
            nc.vector.memset(cc[:], 0.0)
            nc.vector.memset(last[:], 0.0)

            for t in range(Lc):
                et2 = ein.tile([128, nl], BF, tag="et2", name="et2")
                nc.sync.dma_start(et2[0:E, :], d_eT.ap()[t])
                nc.sync.dma_start(et2[E:128, :], d_eT.ap()[t])
                masked = t >= lmin - 1
                hprev = hh[t % 2]
                hcur = hh[(t + 1) % 2]
                for ci in range(NCH):
                    cs = slice(ci * CH, (ci + 1) * CH)
                    if masked:
                        mk = ctmp.tile([128, CH], U8, tag="mk", name="mk")
                        nc.gpsimd.tensor_scalar(mk[:], lenr[:, cs], float(t),
                                                None,
                                                op0=mybir.AluOpType.is_gt)
                    for j in range(2):
                        ps = [cps.tile([128, CH], FP, tag="ps", name="ps")
                              for _ in range(4)]
                        # x-part: two K=64 row strips per PE pass, plus
                        # bias-free accumulation of the two h chunks;
                        # matmul outputs are split into 512-col halves
                        # (one PSUM bank each)
                        for hw_ in range(2):
                            o5 = slice(hw_ * 512, (hw_ + 1) * 512)
                            c5 = slice(ci * CH + hw_ * 512,
                                       ci * CH + (hw_ + 1) * 512)
                            for pi in range(2):
                                sl = 2 * j + pi
                                nc.tensor.matmul(ps[2 * pi][:, o5],
                                                 cWx2[0:E, sl, :],
                                                 et2[0:E, c5],
                                                 start=True, stop=(t == 0))
                                nc.tensor.matmul(ps[2 * pi + 1][:, o5],
                                                 cWx2[E:128, sl, :],
                                                 et2[E:128, c5],
                                                 start=True, stop=(t == 0))
                            if t > 0:
                                for gi in range(4):
                                    m = 2 * gi + j
                                    for k in range(2):
                                        nc.tensor.matmul(
                                            ps[gi][:, o5],
                                            cWh[:, k, m * 128:(m + 1) * 128],
                                            hprev[:, k, c5],
                                            start=False, stop=(k == 1))
                        gsb = [ctmp.tile([128, CH], BF, tag=f"g{gi}",
                                         name=f"g{gi}") for gi in range(4)]
                        for gi in range(4):
                            m = 2 * gi + j
                            nc.scalar.activation(gsb[gi][:], ps[gi][:],
                                                 GF[gi], bias=cb[:, m:m + 1])
                        ig = ctmp.tile([128, CH], BF, tag="ig", name="ig")
                        nc.vector.tensor_mul(ig[:], gsb[0][:], gsb[2][:])
                        fc = ctmp.tile([128, CH], BF, tag="fc", name="fc")
                        nc.vector.tensor_mul(fc[:], gsb[1][:], cc[:, j, cs])
                        nc.vector.tensor_add(cc[:, j, cs], fc[:], ig[:])
                        tct = ctmp.tile([128, CH], BF, tag="tct", name="tct")
                        nc.scalar.activation(tct[:], cc[:, j, cs], TanhF)
                        nc.vector.tensor_mul(hcur[:, j, cs], gsb[3][:],
                                             tct[:])
                        if masked:
                            nc.vector.copy_predicated(last[:, j, cs], mk[:],
                                                      hcur[:, j, cs])
            for j in range(2):
                nc.sync.dma_start(d_last.ap()[j], last[:, j, :])
    nc.compile()
    return nc


def build_l2(bl2=32, fp8=True):
    """Word LSTM v3: fp8 DoubleRow via half-pad windows, step-major."""
    nl = bl2 * S
    nc = bacc.Bacc("TRN2", target_bir_lowering=False, debug=False,
                   num_devices=NCORE)
    d_last = nc.dram_tensor("lastT2", [2, 128, nl], BF, kind="ExternalInput")
    d_wIT = nc.dram_tensor("wIT", [2, 128, WG], BF, kind="ExternalInput")
    d_wb = nc.dram_tensor("wb", [1, WG], BF, kind="ExternalInput")
    d_ones = nc.dram_tensor("onesr", [1, 128], BF, kind="ExternalInput")
    d_eyeb = nc.dram_tensor("eyeb", [128, 32], BF, kind="ExternalInput")
    d_scl = nc.dram_tensor("scl64", [64, 1], FP, kind="ExternalInput")
    d_wh = nc.dram_tensor("wh8", [2, 128, 2, WG], F8T, kind="ExternalInput")
    d_hs = nc.dram_tensor("hsTh", [4, 128, nl], BF, kind="ExternalOutput")
    NT = nl // 128
    IdF = mybir.ActivationFunctionType.Identity

    with tile.TileContext(nc) as tc:
        with ExitStack() as c2:
            ww = c2.enter_context(tc.tile_pool(name="wweights", bufs=1))
            wst = c2.enter_context(tc.tile_pool(name="wstate", bufs=1))
            wtmp = c2.enter_context(tc.tile_pool(name="wtmp", bufs=3))
            eyeb = ww.tile([128, 32], BF, tag="eyeb", name="eyeb")
            nc.sync.dma_start(eyeb[:], d_eyeb.ap()[:])
            ones = ww.tile([1, 128], BF, tag="ones", name="ones")
            nc.sync.dma_start(ones[:], d_ones.ap()[:])
            wbt = ww.tile([1, WG], BF, tag="wbt", name="wbt")
            nc.sync.dma_start(wbt[:], d_wb.ap()[:])
            scl = ww.tile([64, 1], FP, tag="scl", name="scl")
            nc.sync.dma_start(scl[:], d_scl.ap()[:])
            wh = ww.tile([128, 2, 2, WG], F8T, tag="wh", name="wh")
            nc.sync.dma_start(wh[:],
                              d_wh.ap().rearrange("q p i g -> p q i g"))
            xt = wst.tile([128, NT, WG], BF, tag="xt", name="xt")
            hsT = wst.tile([128, 4, S, bl2], BF, tag="hsT", name="hsT")

            lw = c2.enter_context(tc.tile_pool(name="lw", bufs=1))
            psA = c2.enter_context(tc.tile_pool(name="psA", bufs=2,
                                                space="PSUM"))
            lpool = c2.enter_context(tc.tile_pool(name="lpool", bufs=3))
            wIT = lw.tile([128, 2, WG], BF, tag="wIT", name="wIT")
            nc.sync.dma_start(wIT[:],
                              d_wIT.ap().rearrange("k p g -> p k g"))

            def emit_a(tt):
                ts = slice(tt * 128, (tt + 1) * 128)
                lt = lpool.tile([128, 2, 128], BF, tag="lt", name="lt")
                for j2 in range(2):
                    nc.sync.dma_start(lt[:, j2, :], d_last.ap()[j2][:, ts])
                for nch in range(4):
                    sl = slice(nch * 512, (nch + 1) * 512)
                    px = psA.tile([128, 512], FP, tag="px", name="px")
                    nc.tensor.matmul(px[:], ones[:, 0:128], wbt[:, sl],
                                     start=True, stop=False)
                    for j2 in range(2):
                        nc.tensor.matmul(px[:], lt[:, j2, :], wIT[:, j2, sl],
                                         start=False, stop=(j2 == 1))
                    if nch < 2:
                        nc.vector.tensor_scalar(xt[:, tt, sl], px[:], 16.0,
                                                None,
                                                op0=mybir.AluOpType.mult)
                    else:
                        nc.scalar.activation(xt[:, tt, sl], px[:], IdF,
                                             scale=16.0)

            APRE = 3
            for tt in range(APRE):
                emit_a(tt)

            wps = c2.enter_context(tc.tile_pool(name="wpsum", bufs=2,
                                                space="PSUM"))
            wpt = c2.enter_context(tc.tile_pool(name="wpt", bufs=2,
                                                space="PSUM"))
            rgp = c2.enter_context(tc.tile_pool(name="wring", bufs=1))
            # fp8 ring: h^T lives at cols 32-63 of a zero-padded window
            # tile; shifted 64-wide windows stack two gates per DR output
            rlist = []
            for ri in range(3):
                rt = rgp.tile([128, 4, 96], F8T, tag=f"r{ri}",
                              name=f"r{ri}")
                nc.vector.memset(rt[:], 0.0)
                rlist.append(rt)
            c32 = wst.tile([32, 512], BF, tag="c32", name="c32")
            nc.vector.memset(c32[:], 0.0)

            banks = {}

            def emit_inject(s):
                tt, so = divmod(s, 4)
                rs = slice(32 * so, 32 * so + 32)
                pA = wps.tile([64, 512], FP, tag="pA", name="pA")
                pB = wps.tile([64, 512], FP, tag="pB", name="pB")
                banks[s] = (pA, pB)
                for ti, pt_ in ((0, pA), (1, pB)):
                    for half in range(2):
                        g4 = (2 * ti + half) * 512
                        nc.tensor.matmul(pt_[32 * half:32 * half + 32, :],
                                         eyeb[rs, :],
                                         xt[rs, tt, g4:g4 + 512],
                                         start=True, stop=(s == 0),
                                         tile_position=(32 * so, 32 * half),
                                         skip_group_check=True)

            emit_inject(0)
            for s in range(S):
                tt, so = divmod(s, 4)
                if so == 0 and tt + APRE < NT:
                    emit_a(tt + APRE)
                pA, pB = banks.pop(s)
                ring = rlist[s % 3]
                nring = rlist[(s + 1) % 3]
                if s > 0:
                    for ti, pt_ in ((0, pA), (1, pB)):
                        for q in range(2):
                            for half in range(2):
                                g4 = (2 * ti + half) * 512
                                win = slice(32, 96) if half == 0 else \
                                    slice(0, 64)
                                nc.tensor.matmul(
                                    pt_[:], ring[:, 2 * q:2 * q + 2, win],
                                    wh[:, q, :, g4:g4 + 512],
                                    perf_mode=DR, start=False,
                                    stop=(q == 1 and half == 1),
                                    skip_group_check=True)
                if s + 1 < S:
                    emit_inject(s + 1)
                # acts: pA = (f|i) sigmoid; pB = (o|g') sigmoid with the
                # g strip at 2x scale (tanh(x) = 2*sigmoid(2x)-1)
                fi = wtmp.tile([64, 512], BF, tag="fi", name="fi")
                og = wtmp.tile([64, 512], BF, tag="og", name="og")
                nc.scalar.activation(fi[:], pA[:], Sig, scale=1.0 / 16.0)
                nc.scalar.activation(og[:], pB[:], Sig, scale=scl[:])
                g0 = wtmp.tile([64, 512], BF, tag="g0", name="g0")
                nc.vector.tensor_scalar(g0[32:64, :], og[32:64, :], 2.0,
                                        -1.0, op0=mybir.AluOpType.mult,
                                        op1=mybir.AluOpType.add)
                fc = wtmp.tile([32, 512], BF, tag="fc", name="fc")
                nc.vector.tensor_mul(fc[:], fi[0:32, :], c32[:])
                ig = wtmp.tile([32, 512], BF, tag="ig", name="ig")
                nc.vector.tensor_mul(ig[:], fi[32:64, :], g0[32:64, :])
                nc.vector.tensor_add(c32[:], fc[:], ig[:])
                # transposed tail, all inputs at base partition 0
                co = wpt.tile([128, 2, 4, bl2], BF, tag="co", name="co")
                for kk in range(4):
                    nc.tensor.transpose(co[:, 0, kk, :],
                                        c32[:, kk * 128:(kk + 1) * 128],
                                        eyeb[0:32, 0:bl2])
                    nc.tensor.transpose(co[:, 1, kk, :],
                                        og[0:32, kk * 128:(kk + 1) * 128],
                                        eyeb[0:32, 0:bl2])
                tct = wtmp.tile([128, 4, bl2], BF, tag="tct", name="tct")
                nc.scalar.activation(tct[:], co[:, 0, :, :], TanhF)
                nc.vector.tensor_mul(nring[:, :, 32:64], tct[:],
                                     co[:, 1, :, :])
                nc.vector.tensor_mul(hsT[:, :, s, :], tct[:],
                                     co[:, 1, :, :])
            nc.sync.dma_start(
                d_hs.ap().rearrange("k p (s b) -> p k s b", b=bl2), hsT[:])
    nc.compile()
    return nc


def build_l3(bl=BL):
    """MLP + log_softmax, data-parallel."""
    nl = bl * S
    nc = bacc.Bacc("TRN2", target_bir_lowering=False, debug=False,
                   num_devices=NCORE)
    d_hs = nc.dram_tensor("hsT8", [8, 128, nl], BF, kind="ExternalInput")
    d_W1T = nc.dram_tensor("W1T", [8, 128, 256], BF, kind="ExternalInput")
    d_b1 = nc.dram_tensor("b1m", [128, 2], FP, kind="ExternalInput")
    d_W2T = nc.dram_tensor("W2T", [2, 128, 256], BF, kind="ExternalInput")
    d_b2 = nc.dram_tensor("b2m", [128, 2], FP, kind="ExternalInput")
    d_W3T = nc.dram_tensor("W3T", [2, 128, OUT], BF, kind="ExternalInput")
    d_b3 = nc.dram_tensor("b3m", [OUT, 1], FP, kind="ExternalInput")
    d_eye = nc.dram_tensor("eye", [128, 128], FP, kind="ExternalInput")
    d_y = nc.dram_tensor("y", [nl, OUT], FP, kind="ExternalOutput")

    CH = min(512, nl)
    NCH = (nl + CH - 1) // CH

    with tile.TileContext(nc) as tc:
        with ExitStack() as c3:
            mw = c3.enter_context(tc.tile_pool(name="mweights", bufs=1))
            mact = c3.enter_context(tc.tile_pool(name="mact", bufs=1))
            mtmp = c3.enter_context(tc.tile_pool(name="mtmp", bufs=4))
            mps = c3.enter_context(tc.tile_pool(name="mpsum", bufs=2,
                                                space="PSUM"))
            sps = c3.enter_context(tc.tile_pool(name="spsum", bufs=2,
                                                space="PSUM"))
            eye_sb = mw.tile([128, 128], FP, tag="eye", name="eye")
            nc.sync.dma_start(eye_sb[:], d_eye.ap()[:])
            W1 = mw.tile([128, 8, 256], BF, tag="W1", name="W1")
            W2 = mw.tile([128, 2, 256], BF, tag="W2", name="W2")
            W3 = mw.tile([128, 2, OUT], BF, tag="W3", name="W3")
            b1 = mw.tile([128, 2], FP, tag="b1", name="b1")
            b2 = mw.tile([128, 2], FP, tag="b2", name="b2")
            b3 = mw.tile([OUT, 1], FP, tag="b3", name="b3")
            nc.sync.dma_start(W1[:], d_W1T.ap().rearrange("k p g -> p k g"))
            nc.sync.dma_start(W2[:], d_W2T.ap().rearrange("k p g -> p k g"))
            nc.sync.dma_start(W3[:], d_W3T.ap().rearrange("k p g -> p k g"))
            nc.sync.dma_start(b1[:], d_b1.ap()[:])
            nc.sync.dma_start(b2[:], d_b2.ap()[:])
            nc.sync.dma_start(b3[:], d_b3.ap()[:])
            hsT = [mw.tile([128, nl], BF, tag=f"hsT{k}", name=f"hsT{k}")
                   for k in range(8)]
            for k in range(8):
                nc.sync.dma_start(hsT[k][:], d_hs.ap()[k])
            h1 = [mact.tile([128, nl], BF, tag=f"h1{m}", name=f"h1{m}")
                  for m in range(2)]
            h2 = [mact.tile([128, nl], BF, tag=f"h2{m}", name=f"h2{m}")
                  for m in range(2)]
            for ci in range(NCH):
                cs = slice(ci * CH, (ci + 1) * CH)
                for m in range(2):
                    p = mps.tile([128, CH], FP, tag="mp1", name="mp1")
                    for k in range(8):
                        nc.tensor.matmul(
                            p[:], W1[:, k, m * 128:(m + 1) * 128],
                            hsT[k][:, cs], start=(k == 0), stop=(k == 7))
                    nc.scalar.activation(h1[m][:, cs], p[:], ReluF,
                                         bias=b1[:, m:m + 1])
            for ci in range(NCH):
                cs = slice(ci * CH, (ci + 1) * CH)
                for m in range(2):
                    p = mps.tile([128, CH], FP, tag="mp2", name="mp2")
                    for k in range(2):
                        nc.tensor.matmul(
                            p[:], W2[:, k, m * 128:(m + 1) * 128],
                            h1[k][:, cs], start=(k == 0), stop=(k == 1))
                    nc.scalar.activation(h2[m][:, cs], p[:], ReluF,
                                         bias=b2[:, m:m + 1])
            # two passes batched by ACT function: all Exp, then all Ln,
            # so the Exp/Ln activation tables load once each instead of
            # per position-tile
            npt = max(1, nl // 128)
            lgs = [mact.tile([128, OUT], FP, tag=f"lgs{pi}", name=f"lgs{pi}")
                   for pi in range(npt)]
            nmxs = [mact.tile([128, 1], FP, tag=f"nmx{pi}", name=f"nmx{pi}")
                    for pi in range(npt)]
            sms = [mact.tile([128, 1], FP, tag=f"sm{pi}", name=f"sm{pi}")
                   for pi in range(npt)]
            for pi in range(npt):
                pcount = min(128, nl - pi * 128)
                psl = slice(pi * 128, pi * 128 + pcount)
                lg = mps.tile([OUT, pcount], FP, tag="mp3", name="mp3")
                for k in range(2):
                    nc.tensor.matmul(lg[:], W3[:, k, :], h2[k][:, psl],
                                     start=(k == 0), stop=(k == 1))
                lgb = mtmp.tile([OUT, pcount], FP, tag="lgb", name="lgb")
                nc.scalar.activation(lgb[:], lg[:], IdentF, bias=b3[:, 0:1])
                lgr = sps.tile([pcount, OUT], FP, tag="lgr", name="lgr")
                nc.tensor.transpose(lgr[:], lgb[:], eye_sb[0:OUT, 0:OUT])
                nc.vector.tensor_reduce(nmxs[pi][0:pcount, :], lgr[:],
                                        axis=mybir.AxisListType.X,
                                        op=mybir.AluOpType.max, negate=True)
                ex = mtmp.tile([pcount, OUT], FP, tag="ex", name="ex")
                nc.scalar.activation(ex[:], lgr[:], ExpF,
                                     bias=nmxs[pi][0:pcount, :],
                                     accum_out=sms[pi][0:pcount, :])
                nc.vector.tensor_copy(lgs[pi][0:pcount, :], lgr[:])
            for pi in range(npt):
                pcount = min(128, nl - pi * 128)
                psl = slice(pi * 128, pi * 128 + pcount)
                lsm = mtmp.tile([pcount, 1], FP, tag="lsm", name="lsm")
                nc.scalar.activation(lsm[:], sms[pi][0:pcount, :], LnF)
                shift = mtmp.tile([pcount, 1], FP, tag="shift", name="shift")
                nc.vector.tensor_sub(shift[:], nmxs[pi][0:pcount, :], lsm[:])
                yt = mtmp.tile([pcount, OUT], FP, tag="yt", name="yt")
                nc.vector.tensor_scalar(yt[:], lgs[pi][0:pcount, :],
                                        shift[:], None,
                                        op0=mybir.AluOpType.add)
                nc.sync.dma_start(d_y.ap()[psl, :], yt[:])
    nc.compile()
    return nc


def _prep_shared(inputs):
    f32 = np.float32
    cWxT = np.asarray(inputs["cW_ih"], f32).T
    cWx2 = np.zeros((128, 4, 128), f32)
    for j in range(2):
        for pi in range(2):
            cWx2[0:E, 2 * j + pi] = cWxT[:, (j + 4 * pi) * 128:
                                         (j + 4 * pi) * 128 + 128]
            cWx2[E:128, 2 * j + pi] = cWxT[:, (2 + j + 4 * pi) * 128:
                                           (2 + j + 4 * pi) * 128 + 128]
    cWhT = np.ascontiguousarray(
        np.asarray(inputs["cW_hh"], f32).T).reshape(2, 128, G4)
    cbias = (np.asarray(inputs["cb_ih"], f32)
             + np.asarray(inputs["cb_hh"], f32))
    cbias_m = np.ascontiguousarray(cbias.reshape(G4 // 128, 128).T)
    wW, wb = [], []
    for pre in ("f", "b"):
        wih = np.asarray(inputs[pre + "W_ih"], f32)
        whh = np.asarray(inputs[pre + "W_hh"], f32)
        wW.append(np.ascontiguousarray(
            np.concatenate([wih.T, whh.T], 0)).reshape(6, 128, WG))
        wb.append((np.asarray(inputs[pre + "b_ih"], f32)
                   + np.asarray(inputs[pre + "b_hh"], f32)).reshape(1, WG))
    W1T = np.ascontiguousarray(
        np.asarray(inputs["W1"], f32).T.astype(BF_NP)).reshape(8, 128, 256)
    b1m = np.ascontiguousarray(np.asarray(inputs["b1"], f32).reshape(2, 128).T)
    W2T = np.ascontiguousarray(
        np.asarray(inputs["W2"], f32).T.astype(BF_NP)).reshape(2, 128, 256)
    b2m = np.ascontiguousarray(np.asarray(inputs["b2"], f32).reshape(2, 128).T)
    W3T = np.ascontiguousarray(
        np.asarray(inputs["W3"], f32).T.astype(BF_NP)).reshape(2, 128, OUT)
    b3m = np.ascontiguousarray(np.asarray(inputs["b3"], f32).reshape(OUT, 1))
    eye = np.eye(128, dtype=f32)
    onesr = np.ones((1, 128), f32)
    return dict(cWx2=cWx2.astype(BF_NP), cWhT=cWhT.astype(BF_NP),
                cbias=cbias_m, wW=wW, wb=wb, W1T=W1T,
                b1m=b1m, W2T=W2T, b2m=b2m, W3T=W3T, b3m=b3m, eye=eye,
                onesr=onesr)


def _l1_maps(inputs, sh, bl, ncores):
    x = np.asarray(inputs["x"])
    emb = np.asarray(inputs["emb"], np.float32).astype(BF_NP)
    nl = bl * S
    maps = []
    for c in range(ncores):
        xc = x[c * bl:(c + 1) * bl].reshape(nl, Lc)
        lengths = (xc != 0).sum(axis=1).astype(np.float32)
        lenrep = np.ascontiguousarray(
            np.broadcast_to(lengths[None, :].astype(BF_NP), (128, nl)))
        eT = np.ascontiguousarray(emb[xc].transpose(1, 2, 0))
        maps.append(dict(eT=eT, lenrep=lenrep, cWx2=sh["cWx2"],
                         cWhT=sh["cWhT"], cbias=sh["cbias"]))
    return maps


@functools.lru_cache(maxsize=4)
def _modules(bl, lmin=18):
    return build_l1(bl, lmin), build_l2(32), build_l3(bl)


def _gate_blocks(w, order):
    h4 = w.shape[0] // 4
    return np.concatenate([w[g * h4:(g + 1) * h4] for g in order], axis=0)


ORD = (1, 0, 3, 2)   # PyTorch (i,f,g,o) -> (f,i,o,g)
F8_NP = ml_dtypes.float8_e4m3


def _prep_l2(inputs):
    f32 = np.float32
    wIT, wb, wh8 = [], [], []
    for pre in ("f", "b"):
        wih = _gate_blocks(np.asarray(inputs[pre + "W_ih"], f32), ORD)
        whh = _gate_blocks(np.asarray(inputs[pre + "W_hh"], f32), ORD)
        wIT.append(np.ascontiguousarray(wih.T.astype(BF_NP)).reshape(
            2, 128, 4 * H))
        wb.append((_gate_blocks(np.asarray(inputs[pre + "b_ih"], f32), ORD)
                   + _gate_blocks(np.asarray(inputs[pre + "b_hh"], f32),
                                  ORD)).reshape(1, 4 * H).astype(BF_NP))
        whhT = whh.T * 16.0
        wh8.append(np.ascontiguousarray(
            whhT.reshape(2, 2, 128, 4 * H).transpose(0, 2, 1, 3)
            .astype(F8_NP)))
    eyeb = np.zeros((128, 32), f32)
    for p in range(128):
        eyeb[p, p % 32] = 1.0
    scl64 = np.full((64, 1), 1.0 / 16.0, f32)
    scl64[32:64] = 2.0 / 16.0
    return dict(wIT=wIT, wb=wb, wh8=wh8, eyeb=eyeb.astype(BF_NP),
                scl64=scl64, onesr=np.ones((1, 128), f32).astype(BF_NP))


def _l2_maps_v3(last_full, sh2, ncores):
    maps = []
    half = ncores // 2
    for c in range(ncores):
        d = 0 if c < half else 1
        g = c % half
        lt = last_full[:, :, g * 32 * S:(g + 1) * 32 * S].reshape(
            2, 128, 32, S)
        if d == 1:
            lt = lt[:, :, :, ::-1]
        lt = np.ascontiguousarray(
            lt.transpose(0, 1, 3, 2).reshape(2, 128, 32 * S)).astype(BF_NP)
        maps.append(dict(lastT2=lt, wIT=sh2["wIT"][d], wb=sh2["wb"][d],
                         onesr=sh2["onesr"], eyeb=sh2["eyeb"],
                         scl64=sh2["scl64"], wh8=sh2["wh8"][d]))
    return maps


def _l3_maps_v3(hs_f, hs_b, sh, bl, ncores):
    nl = bl * S
    maps = []
    for c in range(ncores):
        g, hf = c // 2, c % 2
        sl = slice(hf * nl, (hf + 1) * nl)
        hs8 = np.concatenate([hs_f[g][:, :, sl], hs_b[g][:, :, sl]], axis=0)
        maps.append(dict(hsT8=np.ascontiguousarray(hs8), W1T=sh["W1T"],
                         b1m=sh["b1m"], W2T=sh["W2T"], b2m=sh["b2m"],
                         W3T=sh["W3T"], b3m=sh["b3m"], eye=sh["eye"]))
    return maps


def _pipeline(inputs, bl, ncores, run_l1, run_l2, run_l3):
    sh = _prep_shared(inputs)
    sh2 = _prep_l2(inputs)
    half = ncores // 2

    r1 = run_l1(_l1_maps(inputs, sh, bl, ncores))
    last_full = np.concatenate(
        [np.asarray(r1[c]["lastT"]).astype(np.float32)
         for c in range(ncores)], axis=2)

    r2 = run_l2(_l2_maps_v3(last_full, sh2, ncores))
    hs_f, hs_b = [], []
    for g in range(half):
        hs_f.append(np.asarray(r2[g]["hsTh"]))
        hb = np.asarray(r2[half + g]["hsTh"]).reshape(4, 128, S, 32)
        hs_b.append(np.ascontiguousarray(
            hb[:, :, ::-1, :]).reshape(4, 128, 32 * S))

    r3 = run_l3(_l3_maps_v3(hs_f, hs_b, sh, bl, ncores))
    out = np.empty((B, S, OUT), np.float32)
    for c in range(ncores):
        y = np.asarray(r3[c]["y"]).reshape(S // 2, 32, OUT)
        bs = 32 * (c // 2)
        ss = (S // 2) * (c % 2)
        out[bs:bs + 32, ss:ss + S // 2] = y.transpose(1, 0, 2)
    return out


def kernel(**inputs):
    x = np.asarray(inputs["x"])
    lmin = int((x.reshape(-1, Lc) != 0).sum(axis=1).min())
    l1, l2, l3 = _modules(BL, lmin)

    def runner(nc):
        def run(in_maps):
            res = bass_utils.run_bass_kernel_spmd(
                nc, in_maps, core_ids=list(range(NCORE)))
            return res.results
        return run

    return _pipeline(inputs, BL, NCORE, runner(l1), runner(l2), runner(l3))
